# revision 21
# baseline (speedup 1.0000x reference)
"""MetaNCA Trainium2 kernel: out = softmax(X @ (W + MLP_percell(W))).

Strategy (8 NeuronCores, SPMD):
  - W row-sharded (256 rows/core) as 126-row tiles; partitions 126/127 hold
    the colsum row and a ones row, so MLP layer 1 is ONE K=128 matmul per
    12-row sub-chunk: pre1 = alpha_c*w + beta_c*colsum_j + (gamma_c*rowsum_i
    + b1_c). Colsum = fp32r masked ones-matmul over the own shard + bf16
    ones-matmuls over the other cores' rows streamed as bf16 (7 MiB) — no
    collective in the critical path (the first-collective fabric wall is
    ~80us here, far more than the bf16 stream).
  - Hidden layers: block-diagonal W2; layer 3 scatters updates tile-wide in
    PSUM; one DVE add produces newW. +b3 dropped (softmax-shift-invariant).
  - MLP chains ordered j-OUTER (columns): each 512-col j-slice of newW
    completes across all tiles, is staged to DRAM and AllGathered immediately
    (jt column-split AGs pipelined under the rest of the MLP + phase 3).
  - Phase 3 single pass per j-block: one [128,512] PSUM per batch-tile
    accumulates fp32r high-X matmuls over 16 k-tiles PLUS the low-X
    correction as 8 fp8e5m2 DoubleRow matmuls (newW cast to fp8 on ACT),
    then lands in SBUF logits; softmax runs in-place per batch-tile right
    after its last j-block, overlapping remaining matmuls. X is
    batch-sharded, host-split into fp32r high + fp8 low parts.
"""

import os
import sys

import numpy as np

for _p in ("/opt/trn_rl_repo", "/root/.axon_site/_ro/trn_rl_repo"):
    if os.path.isdir(_p) and _p not in sys.path:
        sys.path.insert(0, _p)

import ml_dtypes  # noqa: E402

import concourse.bass as bass  # noqa: E402
import concourse.tile as tile  # noqa: E402
from concourse import bacc, bass_utils, mybir  # noqa: E402

F32 = mybir.dt.float32
F32R = mybir.dt.float32r
FP8 = mybir.dt.float8e5
BF16 = mybir.dt.bfloat16
AF = mybir.ActivationFunctionType
Alu = mybir.AluOpType
DR = mybir.MatmulPerfMode.DoubleRow
H = 10
RW = 126  # real W rows per tile (126/127 = colsum/ones)


def _tile_plan(n_shard):
    plan = []
    r = 0
    while r + RW <= n_shard:
        plan.append(RW)
        r += RW
    if r < n_shard:
        plan.append(n_shard - r)
    return plan


def _subchunks(rows):
    subs = []
    r = 0
    while r < rows:
        g = min(12, rows - r)
        subs.append((r, g))
        r += g
    return subs


def build_consts(W1, b1, W2, b2, W3, n, m, n_shard):
    alpha = (W1[0] - W1[1] / np.float32(n - 1) - W1[2] / np.float32(m - 1)).astype(np.float32)
    beta = (W1[1] / np.float32(n - 1)).astype(np.float32)
    gamma = (W1[2] / np.float32(m - 1)).astype(np.float32)
    plan = _tile_plan(n_shard)

    def selb(rows):
        t = np.zeros((128, rows * H), dtype=np.float32)
        for r in range(rows):
            t[r, r * H : (r + 1) * H] = alpha
        t[126, :] = np.tile(beta, rows)
        return t

    def w3sc(rows):
        subs = _subchunks(rows)
        t = np.zeros((120, len(subs) * 128), dtype=np.float32)
        for s, (r0, g_) in enumerate(subs):
            for g in range(g_):
                t[g * H : (g + 1) * H, s * 128 + r0 + g] = W3[:, 0]
        return t

    def blkdiag(mat, g_):
        out = np.zeros((g_ * mat.shape[0], g_ * mat.shape[1]), dtype=np.float32)
        for g in range(g_):
            out[g * mat.shape[0] : (g + 1) * mat.shape[0],
                g * mat.shape[1] : (g + 1) * mat.shape[1]] = mat
        return out

    gset = sorted({g for rows in set(plan) for _, g in _subchunks(rows)})
    c = {
        "ident": np.eye(128, dtype=np.float32),
        "csmaskW": np.concatenate([np.ones(126, np.float32), np.zeros(2, np.float32)])[:, None],
        "ones16": np.ones((128, 1), dtype=ml_dtypes.bfloat16),
    }
    for rows in sorted(set(plan)):
        c[f"selb{rows}"] = selb(rows)
        c[f"w3sc{rows}"] = w3sc(rows)
        c[f"gamT{rows}"] = np.tile(gamma, rows)[None, :]
        c[f"b1T{rows}"] = np.tile(b1, rows)[None, :]
    for g_ in gset:
        c[f"w2b{g_}"] = blkdiag(W2, g_)
        c[f"b2t{g_}"] = np.tile(b2, g_)[:, None].astype(np.float32)
    return c


def build_program(B, N, M, n_cores, xsplit=True):
    n_shard = N // n_cores
    b_shard = B // n_cores
    plan = _tile_plan(n_shard)
    nt = len(plan)
    kt_all = N // 128
    bt_all = b_shard // 128
    jt = M // 512
    kt_rest = kt_all - n_shard // 128
    KH = max(2, kt_all // 4)          # k-tiles per wn sub-tile
    n_wnh = (kt_all + KH - 1) // KH   # wn sub-tiles per j-block

    nc = bacc.Bacc("TRN2", target_bir_lowering=False, debug=False, num_devices=n_cores)

    d = {}
    def din(name, shape, dt):
        d[name] = nc.dram_tensor(name, list(shape), dt, kind="ExternalInput").ap()
    din("wsh", (nt, 128, M), F32R)       # own W shard tiles; row126=0, row127=1
    din("wrest16", (kt_rest, 128, M), BF16)  # other cores' W rows (colsum only)
    din("csmaskW", (128, 1), F32R)
    din("ones16", (128, 1), BF16)
    din("xtc", (N, b_shard), F32R)       # X^T slab (fp32r high part)
    if xsplit:
        din("xtl8", (N, b_shard), FP8)   # fp8 low part of X^T (X - fp32r(X))
    din("ident", (128, 128), F32)
    for rows in sorted(set(plan)):
        din(f"selb{rows}", (128, rows * H), F32R)
        din(f"w3sc{rows}", (120, len(_subchunks(rows)) * 128), F32R)
        din(f"gamT{rows}", (1, rows * H), F32)
        din(f"b1T{rows}", (1, rows * H), F32)
    gset = sorted({g for rows in set(plan) for _, g in _subchunks(rows)})
    for g_ in gset:
        din(f"w2b{g_}", (g_ * H, g_ * H), F32R)
        din(f"b2t{g_}", (g_ * H, 1), F32)
    out_d = nc.dram_tensor("out", [b_shard, M], F32, kind="ExternalOutput").ap()

    rg = [list(range(n_cores))]
    tile_base = []
    acc = 0
    for rows in plan:
        tile_base.append(acc)
        acc += rows

    with tile.TileContext(nc) as tc:
      with tc.tile_pool(name="dram", bufs=1, space="DRAM") as dram:
        agj_in = [dram.tile([n_shard, 512], F32, name=f"agj_in{j}") for j in range(jt)]
        agj = [dram.tile([N, 512], F32, name=f"agj{j}") for j in range(jt)]
        warm_in = dram.tile([1, 16], F32, name="warm_in")
        warm_out = dram.tile([n_cores, 16], F32, name="warm_out")
        with tc.tile_pool(name="wz", bufs=1) as wz:
            wzt = wz.tile([1, 16], F32, name="wzt")
            nc.vector.memset(wzt[:], 0.0)
            nc.sync.dma_start(warm_in[:], wzt[:])
        nc.gpsimd.collective_compute(
            "AllGather", Alu.bypass, ins=[warm_in.opt()], outs=[warm_out.opt()],
            replica_groups=rg)
        xp_ctx = tc.tile_pool(name="xp", bufs=1)
        xp = xp_ctx.__enter__()
        with tc.tile_pool(name="cp", bufs=1) as cp, \
             tc.tile_pool(name="wp", bufs=1) as wp:
            def load(pool, name, dram_ap, shape, dt, eng=None):
                t = pool.tile(shape, dt, name=name)
                (eng or nc.sync).dma_start(t[:], dram_ap[:])
                return t
            # ---- phase 1a: W tiles + full colsum (own fp32r + bf16 stream)
            w_t = []
            for ti, rows in enumerate(plan):
                t = wp.tile([128, M], F32R, name=f"w_t{ti}")
                nc.sync.dma_start(t[:], d["wsh"][ti])
                w_t.append(t)
            csmw_t = load(cp, "csmw_t", d["csmaskW"], [128, 1], F32R, eng=nc.sync)
            ones16_t = load(cp, "ones16_t", d["ones16"], [128, 1], BF16, eng=nc.sync)
            ident_t = load(cp, "ident_t", d["ident"], [128, 128], F32)
            w3_t, w2b_t, b2t_t, gam_t, b1t_t, selbw = {}, {}, {}, {}, {}, []
            for rows in sorted(set(plan)):
                w3_t[rows] = load(cp, f"w3_t{rows}", d[f"w3sc{rows}"],
                                  [120, len(_subchunks(rows)) * 128], F32R)
                gam_t[rows] = load(cp, f"gam_t{rows}", d[f"gamT{rows}"],
                                   [1, rows * H], F32, eng=nc.sync)
                b1t_t[rows] = load(cp, f"b1t_t{rows}", d[f"b1T{rows}"],
                                   [1, rows * H], F32, eng=nc.sync)
            for g_ in gset:
                w2b_t[g_] = load(cp, f"w2b_t{g_}", d[f"w2b{g_}"],
                                 [g_ * H, g_ * H], F32R)
                b2t_t[g_] = load(cp, f"b2t_t{g_}", d[f"b2t{g_}"], [g_ * H, 1], F32,
                                 eng=nc.sync)
            for ti, rows in enumerate(plan):
                st = wp.tile([128, rows * H], F32R, name=f"selbw{ti}", tag=f"selbw{ti}")
                nc.sync.dma_start(st[:], d[f"selb{rows}"][:])
                selbw.append(st)
            with tc.tile_pool(name="p1ps", bufs=1, space="PSUM") as p1ps, \
                 tc.tile_pool(name="p1", bufs=1) as p1, \
                 tc.tile_pool(name="wfp", bufs=1) as wfp:
                def emit_rowsum():
                    rsT_sb = p1.tile([1, nt * 128], F32, name="rsT_sb")
                    for ti in range(nt):
                        rowsum_t = p1.tile([126, 1], F32, name=f"rowsum_{ti}",
                                           tag="rowsum", bufs=2)
                        nc.vector.reduce_sum(rowsum_t[:], w_t[ti][0:126, :].bitcast(F32),
                                             axis=mybir.AxisListType.X)
                        rsT_ps = p1ps.tile([1, 128], F32, name=f"rsT_ps{ti}",
                                           tag="rsT", bufs=2)
                        nc.tensor.transpose(rsT_ps[0:1, 0:126], rowsum_t[:],
                                            ident_t[0:126, 0:126])
                        nc.scalar.copy(rsT_sb[0:1, ti * 128 : ti * 128 + 126],
                                       rsT_ps[0:1, 0:126])
                    for ti, rows in enumerate(plan):
                        r13 = p1.tile([1, rows * H], F32, name=f"r13_{ti}",
                                      tag="r13", bufs=2)
                        rs_b = rsT_sb[0:1, ti * 128 : ti * 128 + rows].unsqueeze(-1) \
                            .broadcast_to([1, rows, H])
                        r3 = r13[:].rearrange("p (n r) -> p n r", r=H)
                        nc.vector.tensor_tensor(
                            r3, rs_b, gam_t[rows][:].rearrange("p (n r) -> p n r", r=H),
                            op=Alu.mult)
                        nc.vector.tensor_tensor(
                            r3, r3, b1t_t[rows][:].rearrange("p (n r) -> p n r", r=H),
                            op=Alu.add)
                        nc.scalar.dma_start(selbw[ti][127:128, :], r13[:].bitcast(F32R))
                CH = 2
                n_ch = (kt_rest + CH - 1) // CH
                wf_t = []
                for c_ in range(n_ch):
                    k0 = c_ * CH
                    kn = min(CH, kt_rest - k0)
                    wf = wfp.tile([128, kn * M], BF16, name=f"wf{c_}", tag="wf", bufs=4)
                    eng = nc.scalar if c_ % 2 == 0 else nc.sync
                    src_ap = d["wrest16"].rearrange("t p m -> p t m")[:, k0 : k0 + kn, :]
                    eng.dma_start(wf[:].rearrange("p (t m) -> p t m", m=M), src_ap)
                    wf_t.append((wf, kn))
                colsum_ps = p1ps.tile([1, M], F32, name="colsum_ps")
                for ti in range(nt):
                    for j in range(jt):
                        sl = slice(j * 512, (j + 1) * 512)
                        nc.tensor.matmul(colsum_ps[:, sl], csmw_t[:], w_t[ti][:, sl],
                                         start=(ti == 0), stop=False)
                emit_rowsum()
                for c_, (wf, kn) in enumerate(wf_t):
                    for g in range(kn):
                        for j in range(jt):
                            sl = slice(g * M + j * 512, g * M + (j + 1) * 512)
                            last = (c_ == n_ch - 1 and g == kn - 1 and j == jt - 1)
                            nc.tensor.matmul(colsum_ps[:, j * 512 : (j + 1) * 512],
                                             ones16_t[:], wf[:, sl],
                                             start=False, stop=last)
                # colsum out of PSUM -> stats row of every W tile
                colsum_sb = p1.tile([1, M], F32, name="colsum_sb")
                nc.scalar.copy(colsum_sb[:], colsum_ps[:])
                for ti in range(nt):
                    nc.scalar.dma_start(w_t[ti][126:127, :], colsum_sb[:].bitcast(F32R))
                # warm exp table
                wdum = p1.tile([1, 8], F32, name="wdum")
                nc.vector.memset(wdum[:], 0.0)
                nc.scalar.activation(wdum[:], wdum[:], AF.Exp)

            # X^T prefetch (DMA is otherwise idle during the MLP)
            xtb_t, xtl_t = {}, {}
            def load_x(bt):
                t = xp.tile([128, kt_all * 128], F32R, name=f"xtb{bt}", tag="xtb",
                            bufs=bt_all)
                src = d["xtc"][:, bt * 128 : (bt + 1) * 128].rearrange(
                    "(kt p) b -> p kt b", p=128)
                nc.sync.dma_start(t[:].rearrange("p (kt b) -> p kt b", b=128), src)
                xtb_t[bt] = t
                if xsplit:
                    tl = xp.tile([128, kt_all * 128], FP8, name=f"xtl{bt}", tag="xtl",
                                 bufs=bt_all)
                    srcl = d["xtl8"][:, bt * 128 : (bt + 1) * 128].rearrange(
                        "(kt p) b -> p kt b", p=128)
                    nc.sync.dma_start(tl[:].rearrange("p (kt b) -> p kt b", b=128), srcl)
                    xtl_t[bt] = tl
            for bt in range(bt_all):
                load_x(bt)

            # ---- phase 2: MLP, j-OUTER; AG per completed j-slice
            with tc.tile_pool(name="nwp", bufs=1) as nwp, \
                 tc.tile_pool(name="hp", bufs=4) as hp, \
                 tc.tile_pool(name="p2ps", bufs=1, space="PSUM") as p2ps:
                chains = []
                for j in range(jt):
                    for ti, rows in enumerate(plan):
                        for s, (r0, g_) in enumerate(_subchunks(rows)):
                            chains.append((ti, j, s, r0, g_, rows))
                nC = len(chains)
                state = {}
                nw_t = {}

                def emit_mm1(c, idx):
                    ti, j, s, r0, g_, rows = c
                    sl = slice(j * 512, (j + 1) * 512)
                    Mh = g_ * H
                    ps1 = p2ps.tile([120, 512], F32, name=f"ps1_{ti}_{j}_{s}", tag="ps1", bufs=3)
                    nc.tensor.matmul(ps1[0:Mh, :], selbw[ti][:, r0 * H : r0 * H + Mh],
                                     w_t[ti][:, sl], start=True, stop=True)
                    h1 = hp.tile([120, 512], F32R, name=f"h1_{ti}_{j}_{s}", tag="h1")
                    if idx % 2 == 0:
                        nc.scalar.activation(h1[0:Mh, :], ps1[0:Mh, :], AF.Relu)
                    else:
                        nc.vector.tensor_scalar(h1[0:Mh, :], ps1[0:Mh, :], 0.0, 0.0,
                                                op0=Alu.add, op1=Alu.max)
                    state[c] = (ps1, h1)

                def emit_mm2(c, idx):
                    ti, j, s, r0, g_, rows = c
                    Mh = g_ * H
                    _, h1 = state[c]
                    ps2 = p2ps.tile([120, 512], F32, name=f"ps2_{ti}_{j}_{s}", tag="ps2", bufs=3)
                    nc.tensor.matmul(ps2[0:Mh, :], w2b_t[g_][:], h1[0:Mh, :], start=True, stop=True)
                    h2 = hp.tile([120, 512], F32R, name=f"h2_{ti}_{j}_{s}", tag="h2")
                    if idx % 2 == 0:
                        nc.vector.tensor_scalar(h2[0:Mh, :], ps2[0:Mh, :], b2t_t[g_][0:Mh, :],
                                                0.0, op0=Alu.add, op1=Alu.max)
                    else:
                        nc.scalar.activation(h2[0:Mh, :], ps2[0:Mh, :], AF.Relu,
                                             bias=b2t_t[g_][0:Mh, :])
                    state[c] = (state[c][0], state[c][1], ps2, h2)

                def emit_mm3(c):
                    ti, j, s, r0, g_, rows = c
                    sl = slice(j * 512, (j + 1) * 512)
                    Mh = g_ * H
                    h2 = state.pop(c)[3]
                    subs = _subchunks(rows)
                    key = (ti, j)
                    if key not in upd_ps:
                        upd_ps[key] = p2ps.tile([128, 512], F32, name=f"upd_{ti}_{j}",
                                                tag="upd", bufs=2)
                    nc.tensor.matmul(upd_ps[key][:], w3_t[rows][0:Mh, s * 128 : (s + 1) * 128],
                                     h2[0:Mh, :], start=(s == 0), stop=(s == len(subs) - 1))
                    if s == len(subs) - 1:
                        if ti not in nw_t:
                            nw_t[ti] = nwp.tile([128, M], F32, name=f"nw_t{ti}", tag=f"nw{ti}")
                        nc.vector.tensor_tensor(nw_t[ti][0:rows, sl], upd_ps[key][0:rows, :],
                                                w_t[ti][0:rows, sl].bitcast(F32), op=Alu.add)
                        del upd_ps[key]
                        nc.scalar.dma_start(
                            agj_in[j][tile_base[ti] : tile_base[ti] + rows, :],
                            nw_t[ti][0:rows, sl])
                        done_t[j] = done_t.get(j, 0) + 1
                        if done_t[j] == nt:
                            nc.gpsimd.collective_compute(
                                "AllGather", Alu.bypass, ins=[agj_in[j].opt()],
                                outs=[agj[j].opt()], replica_groups=rg)

                upd_ps, done_t = {}, {}
                DEPTH = 3
                for i in range(nC + DEPTH):
                    if i < nC:
                        emit_mm1(chains[i], i)
                    if 0 <= i - 1 < nC:
                        emit_mm2(chains[i - 1], i)
                    if 0 <= i - DEPTH < nC:
                        emit_mm3(chains[i - DEPTH])

        # ---- phase 3: big matmul per j-block + per-bt in-place softmax
        with tc.tile_pool(name="wnp", bufs=1) as wnp, \
             tc.tile_pool(name="lgp", bufs=1) as lgp, \
             tc.tile_pool(name="smp", bufs=2) as smp, \
             tc.tile_pool(name="p3ps", bufs=1, space="PSUM") as p3ps:
            lg_sb = {}
            mxp = {}
            for jb in range(jt):
                wn_h, wn8_h = [], []
                for hh in range(n_wnh):
                    k0 = hh * KH
                    kn = min(KH, kt_all - k0)
                    wn = wnp.tile([128, kn * 512], F32R, name=f"wn_{jb}_{hh}",
                                  tag="wn", bufs=6)
                    src = agj[jb][k0 * 128 : (k0 + kn) * 128, :].rearrange(
                        "(kt p) m -> p kt m", p=128).bitcast(F32R)
                    nc.sync.dma_start(wn[:].rearrange("p (kt m) -> p kt m", m=512), src)
                    wn_h.append(wn)
                    if xsplit:
                        wn8 = wnp.tile([128, kn * 512], FP8, name=f"wn8_{jb}_{hh}",
                                       tag="wn8", bufs=6)
                        nc.vector.tensor_copy(wn8[:], wn[:].bitcast(F32))
                        wn8_h.append(wn8[:].rearrange("p (t o m) -> p t o m", o=2, m=512))
                for b0 in range(0, bt_all, 2):
                    pair = [b0, b0 + 1] if b0 + 1 < bt_all else [b0]
                    pss = {}
                    for bt in pair:
                        if jb == 0:
                            lg_sb[bt] = lgp.tile([128, M], F32, name=f"lg{jb}_{bt}",
                                                 tag=f"lg{bt}", bufs=1)
                        pss[bt] = p3ps.tile([128, 512], F32, name=f"ps_{jb}_{bt}",
                                            tag="lgps", bufs=4)
                    # interleave the pair so consecutive matmuls alternate PSUM banks
                    for kt in range(kt_all):
                        rhs = wn_h[kt // KH][:, (kt % KH) * 512 : (kt % KH + 1) * 512]
                        for bt in pair:
                            nc.tensor.matmul(pss[bt][:],
                                             xtb_t[bt][:, kt * 128 : (kt + 1) * 128], rhs,
                                             start=(kt == 0),
                                             stop=(kt == kt_all - 1 and not xsplit))
                    if xsplit:
                        kp = kt_all // 2
                        kpH = KH // 2
                        for t in range(kp):
                            rhs8 = wn8_h[t // kpH][:, t % kpH]
                            for bt in pair:
                                xl3d = xtl_t[bt][:].rearrange("p (t o b) -> p t o b",
                                                              o=2, b=128)
                                nc.tensor.matmul(pss[bt][:], xl3d[:, t], rhs8,
                                                 start=False, stop=(t == kp - 1),
                                                 perf_mode=DR)
                    # close the pair: copy to SBUF logits, partial max, softmax
                    for bt in pair:
                        lg = lg_sb[bt]
                        ps = pss[bt]
                        if bt % 2 == 0:
                            nc.scalar.copy(lg[:, jb * 512 : (jb + 1) * 512], ps[:])
                        else:
                            nc.vector.tensor_copy(lg[:, jb * 512 : (jb + 1) * 512], ps[:])
                        if bt not in mxp:
                            mxp[bt] = lgp.tile([128, jt], F32, name=f"mxp{bt}",
                                               tag=f"mxp{bt}")
                        nc.vector.reduce_max(mxp[bt][:, jb : jb + 1], ps[:],
                                             axis=mybir.AxisListType.X)
                        if jb != jt - 1:
                            continue
                        # softmax(bt) in place, overlapping later bts' matmuls
                        mx = smp.tile([128, 1], F32, name=f"mx{bt}", tag="mx")
                        nc.vector.reduce_max(mx[:], mxp[bt][:], axis=mybir.AxisListType.X)
                        nmx = smp.tile([128, 1], F32, name=f"nmx{bt}", tag="nmx")
                        nc.vector.tensor_scalar_mul(nmx[:], mx[:], -1.0)
                        sume = smp.tile([128, 1], F32, name=f"sume{bt}", tag="sume")
                        nc.scalar.activation(lg[:], lg[:], AF.Exp, bias=nmx[:],
                                             accum_out=sume[:])
                        rec = smp.tile([128, 1], F32, name=f"rec{bt}", tag="rec")
                        nc.vector.reciprocal(rec[:], sume[:])
                        nc.vector.tensor_scalar_mul(lg[:], lg[:], rec[:])
                        nc.gpsimd.dma_start(out_d[bt * 128 : (bt + 1) * 128, :], lg[:])
        xp_ctx.__exit__(None, None, None)

    nc.compile()
    meta = dict(B=B, N=N, M=M, n_cores=n_cores, n_shard=n_shard, b_shard=b_shard,
                plan=plan)
    return nc, meta


_CACHE = {}


def _get_program(B, N, M, n_cores, xsplit=True):
    key = (B, N, M, n_cores, xsplit)
    if key not in _CACHE:
        _CACHE[key] = build_program(B, N, M, n_cores, xsplit)
    return _CACHE[key]


def _round_fp32r(x):
    xi = x.view(np.uint32).astype(np.uint64)
    xi = (xi + (1 << 11)) & np.uint64(0xFFFFF000)
    return xi.astype(np.uint32).view(np.float32)


def make_in_maps(meta, consts, X, weight, xsplit=True):
    n_cores, n_shard, b_shard = meta["n_cores"], meta["n_shard"], meta["b_shard"]
    plan = meta["plan"]
    nt = len(plan)
    M = meta["M"]
    N = n_shard * n_cores
    XTp = np.ascontiguousarray(X.T)
    if xsplit:
        XTh = _round_fp32r(XTp)
        XTl8 = (XTp - XTh).astype(ml_dtypes.float8_e5m2)
        XTp = XTh
    base = dict(consts)
    in_maps = []
    for c in range(n_cores):
        m = dict(base)
        rot = np.concatenate([weight[c * n_shard :], weight[: c * n_shard]], axis=0)
        m["wrest16"] = np.ascontiguousarray(
            rot[n_shard:].astype(ml_dtypes.bfloat16).reshape(
                N // 128 - n_shard // 128, 128, M))
        wt = np.zeros((nt, 128, M), dtype=np.float32)
        acc = 0
        for ti, rows in enumerate(plan):
            wt[ti, 0:rows, :] = weight[c * n_shard + acc : c * n_shard + acc + rows, :]
            wt[ti, 127, :] = 1.0
            acc += rows
        m["wsh"] = wt
        m["xtc"] = np.ascontiguousarray(XTp[:, c * b_shard : (c + 1) * b_shard])
        if xsplit:
            m["xtl8"] = np.ascontiguousarray(XTl8[:, c * b_shard : (c + 1) * b_shard])
        in_maps.append(m)
    return in_maps


def run(X, weight, W1, b1, W2, b2, W3, b3, n_cores=8, trace=False, xsplit=True,
        **hw_kwargs):
    X = np.asarray(X, dtype=np.float32)
    weight = np.asarray(weight, dtype=np.float32)
    B, N = X.shape
    M = weight.shape[1]
    nc, meta = _get_program(B, N, M, n_cores, xsplit)
    consts = build_consts(np.asarray(W1, np.float32), np.asarray(b1, np.float32),
                          np.asarray(W2, np.float32), np.asarray(b2, np.float32),
                          np.asarray(W3, np.float32), N, M, meta["n_shard"])
    in_maps = make_in_maps(meta, consts, X, weight, xsplit=xsplit)
    res = bass_utils.run_bass_kernel_spmd(nc, in_maps, core_ids=list(range(n_cores)),
                                          trace=trace, **hw_kwargs)
    out = np.concatenate([res.results[c]["out"] for c in range(n_cores)], axis=0)
    return out, res


def kernel(X, weight, W1, b1, W2, b2, W3, b3):
    xsplit = os.environ.get("BASSNCA_XSPLIT", "1") != "0"
    out, _ = run(X, weight, W1, b1, W2, b2, W3, b3, xsplit=xsplit)
    return out


# revision 25
# speedup vs baseline: 1.1076x; 1.1076x over previous
"""MetaNCA Trainium2 kernel: out = softmax(X @ (W + MLP_percell(W))).

Strategy (8 NeuronCores, SPMD):
  - W row-sharded (256 rows/core) as 126-row tiles; partitions 126/127 hold
    the colsum row and a ones row, so MLP layer 1 is ONE K=128 matmul per
    12-row sub-chunk: pre1 = alpha_c*w + beta_c*colsum_j + (gamma_c*rowsum_i
    + b1_c). Colsum = fp32r masked ones-matmul over the own shard + bf16
    ones-matmuls over the other cores' rows streamed as bf16 (7 MiB) — no
    collective in the critical path (the first-collective fabric wall is
    ~80us here, far more than the bf16 stream).
  - Hidden layers: block-diagonal W2; layer 3 scatters updates tile-wide in
    PSUM; one DVE add produces newW. +b3 dropped (softmax-shift-invariant).
  - MLP chains ordered j-OUTER (columns): each 512-col j-slice of newW
    completes across all tiles, is staged to DRAM and AllGathered immediately
    (jt column-split AGs pipelined under the rest of the MLP + phase 3).
  - Phase 3 single pass per j-block: one [128,512] PSUM per batch-tile
    accumulates fp32r high-X matmuls over 16 k-tiles PLUS the low-X
    correction as 8 fp8e5m2 DoubleRow matmuls (newW cast to fp8 on ACT),
    then lands in SBUF logits; softmax runs in-place per batch-tile right
    after its last j-block, overlapping remaining matmuls. X is
    batch-sharded, host-split into fp32r high + fp8 low parts.
"""

import os
import sys

import numpy as np

for _p in ("/opt/trn_rl_repo", "/root/.axon_site/_ro/trn_rl_repo"):
    if os.path.isdir(_p) and _p not in sys.path:
        sys.path.insert(0, _p)

import ml_dtypes  # noqa: E402

import concourse.bass as bass  # noqa: E402
import concourse.tile as tile  # noqa: E402
from concourse import bacc, bass_utils, mybir  # noqa: E402

F32 = mybir.dt.float32
F32R = mybir.dt.float32r
FP8 = mybir.dt.float8e5
BF16 = mybir.dt.bfloat16
AF = mybir.ActivationFunctionType
Alu = mybir.AluOpType
DR = mybir.MatmulPerfMode.DoubleRow
H = 10
RW = 126  # real W rows per tile (126/127 = colsum/ones)


def _tile_plan(n_shard):
    plan = []
    r = 0
    while r + RW <= n_shard:
        plan.append(RW)
        r += RW
    if r < n_shard:
        plan.append(n_shard - r)
    return plan


def _subchunks(rows):
    subs = []
    r = 0
    while r < rows:
        g = min(12, rows - r)
        subs.append((r, g))
        r += g
    return subs


def build_consts(W1, b1, W2, b2, W3, n, m, n_shard):
    alpha = (W1[0] - W1[1] / np.float32(n - 1) - W1[2] / np.float32(m - 1)).astype(np.float32)
    beta = (W1[1] / np.float32(n - 1)).astype(np.float32)
    gamma = (W1[2] / np.float32(m - 1)).astype(np.float32)
    plan = _tile_plan(n_shard)

    def selb(rows):
        t = np.zeros((128, rows * H), dtype=np.float32)
        for r in range(rows):
            t[r, r * H : (r + 1) * H] = alpha
        t[126, :] = np.tile(beta, rows)
        return t

    def w3sc(rows):
        subs = _subchunks(rows)
        t = np.zeros((120, len(subs) * 128), dtype=np.float32)
        for s, (r0, g_) in enumerate(subs):
            for g in range(g_):
                t[g * H : (g + 1) * H, s * 128 + r0 + g] = W3[:, 0]
        return t

    def blkdiag(mat, g_):
        out = np.zeros((g_ * mat.shape[0], g_ * mat.shape[1]), dtype=np.float32)
        for g in range(g_):
            out[g * mat.shape[0] : (g + 1) * mat.shape[0],
                g * mat.shape[1] : (g + 1) * mat.shape[1]] = mat
        return out

    gset = sorted({g for rows in set(plan) for _, g in _subchunks(rows)})
    c = {
        "ident": np.eye(128, dtype=np.float32),
        "csmaskW": np.concatenate([np.ones(126, np.float32), np.zeros(2, np.float32)])[:, None],
        "ones16": np.ones((128, 1), dtype=ml_dtypes.bfloat16),
    }
    for rows in sorted(set(plan)):
        c[f"selb{rows}"] = selb(rows)
        c[f"w3sc{rows}"] = w3sc(rows)
        c[f"gamT{rows}"] = np.tile(gamma, rows)[None, :]
        c[f"b1T{rows}"] = np.tile(b1, rows)[None, :]
    for g_ in gset:
        c[f"w2b{g_}"] = blkdiag(W2, g_)
        c[f"b2t{g_}"] = np.tile(b2, g_)[:, None].astype(np.float32)
    return c


def build_program(B, N, M, n_cores, xsplit=True):
    n_shard = N // n_cores
    b_shard = B // n_cores
    plan = _tile_plan(n_shard)
    nt = len(plan)
    kt_all = N // 128
    bt_all = b_shard // 128
    jt = M // 512
    kt_rest = kt_all - n_shard // 128
    KH = max(2, kt_all // 4)          # k-tiles per wn sub-tile
    n_wnh = (kt_all + KH - 1) // KH   # wn sub-tiles per j-block

    nc = bacc.Bacc("TRN2", target_bir_lowering=False, debug=False, num_devices=n_cores)

    d = {}
    def din(name, shape, dt):
        d[name] = nc.dram_tensor(name, list(shape), dt, kind="ExternalInput").ap()
    din("wsh", (nt, 128, M), F32R)       # own W shard tiles; row126=0, row127=1
    din("wrest16", (kt_rest, 128, M), BF16)  # other cores' W rows (colsum only)
    din("csmaskW", (128, 1), F32R)
    din("ones16", (128, 1), BF16)
    din("xtc", (N, b_shard), F32R)       # X^T slab (fp32r high part)
    if xsplit:
        din("xtl8", (N, b_shard), FP8)   # fp8 low part of X^T (X - fp32r(X))
    din("ident", (128, 128), F32)
    for rows in sorted(set(plan)):
        din(f"selb{rows}", (128, rows * H), F32R)
        din(f"w3sc{rows}", (120, len(_subchunks(rows)) * 128), F32R)
        din(f"gamT{rows}", (1, rows * H), F32)
        din(f"b1T{rows}", (1, rows * H), F32)
    gset = sorted({g for rows in set(plan) for _, g in _subchunks(rows)})
    for g_ in gset:
        din(f"w2b{g_}", (g_ * H, g_ * H), F32R)
        din(f"b2t{g_}", (g_ * H, 1), F32)
    out_d = nc.dram_tensor("out", [b_shard, M], F32, kind="ExternalOutput").ap()

    rg = [list(range(n_cores))]
    tile_base = []
    acc = 0
    for rows in plan:
        tile_base.append(acc)
        acc += rows

    with tile.TileContext(nc) as tc:
      with tc.tile_pool(name="dram", bufs=1, space="DRAM") as dram:
        agj_in = [dram.tile([n_shard, 512], F32, name=f"agj_in{j}") for j in range(jt)]
        agj = [dram.tile([N, 512], F32, name=f"agj{j}") for j in range(jt)]
        warm_in = dram.tile([1, 16], F32, name="warm_in")
        warm_out = dram.tile([n_cores, 16], F32, name="warm_out")
        with tc.tile_pool(name="wz", bufs=1) as wz:
            wzt = wz.tile([1, 16], F32, name="wzt")
            nc.vector.memset(wzt[:], 0.0)
            nc.sync.dma_start(warm_in[:], wzt[:])
        nc.gpsimd.collective_compute(
            "AllGather", Alu.bypass, ins=[warm_in.opt()], outs=[warm_out.opt()],
            replica_groups=rg)
        xp_ctx = tc.tile_pool(name="xp", bufs=1)
        xp = xp_ctx.__enter__()
        with tc.tile_pool(name="cp", bufs=1) as cp, \
             tc.tile_pool(name="wp", bufs=1) as wp:
            def load(pool, name, dram_ap, shape, dt, eng=None):
                t = pool.tile(shape, dt, name=name)
                (eng or nc.sync).dma_start(t[:], dram_ap[:])
                return t
            # ---- phase 1a: W tiles + full colsum (own fp32r + bf16 stream)
            w_t = []
            for ti, rows in enumerate(plan):
                t = wp.tile([128, M], F32R, name=f"w_t{ti}")
                nc.sync.dma_start(t[:], d["wsh"][ti])
                w_t.append(t)
            csmw_t = load(cp, "csmw_t", d["csmaskW"], [128, 1], F32R, eng=nc.sync)
            ones16_t = load(cp, "ones16_t", d["ones16"], [128, 1], BF16, eng=nc.sync)
            ident_t = load(cp, "ident_t", d["ident"], [128, 128], F32)
            w3_t, w2b_t, b2t_t, gam_t, b1t_t, selbw = {}, {}, {}, {}, {}, []
            for rows in sorted(set(plan)):
                w3_t[rows] = load(cp, f"w3_t{rows}", d[f"w3sc{rows}"],
                                  [120, len(_subchunks(rows)) * 128], F32R)
                gam_t[rows] = load(cp, f"gam_t{rows}", d[f"gamT{rows}"],
                                   [1, rows * H], F32, eng=nc.sync)
                b1t_t[rows] = load(cp, f"b1t_t{rows}", d[f"b1T{rows}"],
                                   [1, rows * H], F32, eng=nc.sync)
            for g_ in gset:
                w2b_t[g_] = load(cp, f"w2b_t{g_}", d[f"w2b{g_}"],
                                 [g_ * H, g_ * H], F32R)
                b2t_t[g_] = load(cp, f"b2t_t{g_}", d[f"b2t{g_}"], [g_ * H, 1], F32,
                                 eng=nc.sync)
            for ti, rows in enumerate(plan):
                st = wp.tile([128, rows * H], F32R, name=f"selbw{ti}", tag=f"selbw{ti}")
                nc.sync.dma_start(st[:], d[f"selb{rows}"][:])
                selbw.append(st)
            with tc.tile_pool(name="p1ps", bufs=1, space="PSUM") as p1ps, \
                 tc.tile_pool(name="p1", bufs=1) as p1, \
                 tc.tile_pool(name="wfp", bufs=1) as wfp:
                def emit_rowsum():
                    rsT_sb = p1.tile([1, nt * 128], F32, name="rsT_sb")
                    for ti in range(nt):
                        rowsum_t = p1.tile([126, 1], F32, name=f"rowsum_{ti}",
                                           tag="rowsum", bufs=2)
                        nc.vector.reduce_sum(rowsum_t[:], w_t[ti][0:126, :].bitcast(F32),
                                             axis=mybir.AxisListType.X)
                        rsT_ps = p1ps.tile([1, 128], F32, name=f"rsT_ps{ti}",
                                           tag="rsT", bufs=2)
                        nc.tensor.transpose(rsT_ps[0:1, 0:126], rowsum_t[:],
                                            ident_t[0:126, 0:126])
                        nc.scalar.copy(rsT_sb[0:1, ti * 128 : ti * 128 + 126],
                                       rsT_ps[0:1, 0:126])
                    for ti, rows in enumerate(plan):
                        r13 = p1.tile([1, rows * H], F32, name=f"r13_{ti}",
                                      tag="r13", bufs=2)
                        rs_b = rsT_sb[0:1, ti * 128 : ti * 128 + rows].unsqueeze(-1) \
                            .broadcast_to([1, rows, H])
                        r3 = r13[:].rearrange("p (n r) -> p n r", r=H)
                        nc.vector.tensor_tensor(
                            r3, rs_b, gam_t[rows][:].rearrange("p (n r) -> p n r", r=H),
                            op=Alu.mult)
                        nc.vector.tensor_tensor(
                            r3, r3, b1t_t[rows][:].rearrange("p (n r) -> p n r", r=H),
                            op=Alu.add)
                        nc.scalar.dma_start(selbw[ti][127:128, :], r13[:].bitcast(F32R))
                CH = 2
                n_ch = (kt_rest + CH - 1) // CH
                wf_t = []
                for c_ in range(n_ch):
                    k0 = c_ * CH
                    kn = min(CH, kt_rest - k0)
                    wf = wfp.tile([128, kn * M], BF16, name=f"wf{c_}", tag="wf", bufs=4)
                    eng = nc.scalar if c_ % 2 == 0 else nc.sync
                    src_ap = d["wrest16"].rearrange("t p m -> p t m")[:, k0 : k0 + kn, :]
                    eng.dma_start(wf[:].rearrange("p (t m) -> p t m", m=M), src_ap)
                    wf_t.append((wf, kn))
                colsum_ps = p1ps.tile([1, M], F32, name="colsum_ps")
                for ti in range(nt):
                    for j in range(jt):
                        sl = slice(j * 512, (j + 1) * 512)
                        nc.tensor.matmul(colsum_ps[:, sl], csmw_t[:], w_t[ti][:, sl],
                                         start=(ti == 0), stop=False)
                emit_rowsum()
                for c_, (wf, kn) in enumerate(wf_t):
                    for g in range(kn):
                        for j in range(jt):
                            sl = slice(g * M + j * 512, g * M + (j + 1) * 512)
                            last = (c_ == n_ch - 1 and g == kn - 1 and j == jt - 1)
                            nc.tensor.matmul(colsum_ps[:, j * 512 : (j + 1) * 512],
                                             ones16_t[:], wf[:, sl],
                                             start=False, stop=last)
                # colsum out of PSUM -> stats row of every W tile
                colsum_sb = p1.tile([1, M], F32, name="colsum_sb")
                nc.scalar.copy(colsum_sb[:], colsum_ps[:])
                for ti in range(nt):
                    nc.scalar.dma_start(w_t[ti][126:127, :], colsum_sb[:].bitcast(F32R))
                # warm exp table
                wdum = p1.tile([1, 8], F32, name="wdum")
                nc.vector.memset(wdum[:], 0.0)
                nc.scalar.activation(wdum[:], wdum[:], AF.Exp)

            # X^T prefetch (DMA is otherwise idle during the MLP)
            xtb_t, xtl_t = {}, {}
            def load_x(bt):
                t = xp.tile([128, kt_all * 128], F32R, name=f"xtb{bt}", tag="xtb",
                            bufs=bt_all)
                src = d["xtc"][:, bt * 128 : (bt + 1) * 128].rearrange(
                    "(kt p) b -> p kt b", p=128)
                nc.sync.dma_start(t[:].rearrange("p (kt b) -> p kt b", b=128), src)
                xtb_t[bt] = t
                if xsplit:
                    tl = xp.tile([128, kt_all * 128], FP8, name=f"xtl{bt}", tag="xtl",
                                 bufs=bt_all)
                    srcl = d["xtl8"][:, bt * 128 : (bt + 1) * 128].rearrange(
                        "(kt p) b -> p kt b", p=128)
                    nc.sync.dma_start(tl[:].rearrange("p (kt b) -> p kt b", b=128), srcl)
                    xtl_t[bt] = tl
            for bt in range(bt_all):
                load_x(bt)

            # ---- phase 2: MLP, j-OUTER; AG per completed j-slice
            with tc.tile_pool(name="nwp", bufs=1) as nwp, \
                 tc.tile_pool(name="hp", bufs=6) as hp, \
                 tc.tile_pool(name="p2ps", bufs=1, space="PSUM") as p2ps:
                chains = []
                for j in range(jt):
                    for ti, rows in enumerate(plan):
                        for s, (r0, g_) in enumerate(_subchunks(rows)):
                            chains.append((ti, j, s, r0, g_, rows))
                nC = len(chains)
                state = {}
                nw_t = {}

                def emit_mm1(c, idx):
                    ti, j, s, r0, g_, rows = c
                    sl = slice(j * 512, (j + 1) * 512)
                    Mh = g_ * H
                    ps1 = p2ps.tile([120, 512], F32, name=f"ps1_{ti}_{j}_{s}", tag="ps1", bufs=3)
                    nc.tensor.matmul(ps1[0:Mh, :], selbw[ti][:, r0 * H : r0 * H + Mh],
                                     w_t[ti][:, sl], start=True, stop=True)
                    h1 = hp.tile([120, 512], F32R, name=f"h1_{ti}_{j}_{s}", tag="h1")
                    if idx % 2 == 0:
                        nc.scalar.activation(h1[0:Mh, :], ps1[0:Mh, :], AF.Relu)
                    else:
                        nc.vector.tensor_scalar(h1[0:Mh, :], ps1[0:Mh, :], 0.0, 0.0,
                                                op0=Alu.add, op1=Alu.max)
                    state[c] = (ps1, h1)

                def emit_mm2(c, idx):
                    ti, j, s, r0, g_, rows = c
                    Mh = g_ * H
                    _, h1 = state[c]
                    ps2 = p2ps.tile([120, 512], F32, name=f"ps2_{ti}_{j}_{s}", tag="ps2", bufs=3)
                    nc.tensor.matmul(ps2[0:Mh, :], w2b_t[g_][:], h1[0:Mh, :], start=True, stop=True)
                    h2 = hp.tile([120, 512], F32R, name=f"h2_{ti}_{j}_{s}", tag="h2")
                    if idx % 2 == 0:
                        nc.vector.tensor_scalar(h2[0:Mh, :], ps2[0:Mh, :], b2t_t[g_][0:Mh, :],
                                                0.0, op0=Alu.add, op1=Alu.max)
                    else:
                        nc.scalar.activation(h2[0:Mh, :], ps2[0:Mh, :], AF.Relu,
                                             bias=b2t_t[g_][0:Mh, :])
                    state[c] = (state[c][0], state[c][1], ps2, h2)

                def emit_mm3(c):
                    ti, j, s, r0, g_, rows = c
                    sl = slice(j * 512, (j + 1) * 512)
                    Mh = g_ * H
                    h2 = state.pop(c)[3]
                    subs = _subchunks(rows)
                    key = (ti, j)
                    if key not in upd_ps:
                        upd_ps[key] = p2ps.tile([128, 512], F32, name=f"upd_{ti}_{j}",
                                                tag="upd", bufs=2)
                    nc.tensor.matmul(upd_ps[key][:], w3_t[rows][0:Mh, s * 128 : (s + 1) * 128],
                                     h2[0:Mh, :], start=(s == 0), stop=(s == len(subs) - 1))
                    if s == len(subs) - 1:
                        if ti not in nw_t:
                            nw_t[ti] = nwp.tile([128, M], F32, name=f"nw_t{ti}", tag=f"nw{ti}")
                        nc.vector.tensor_tensor(nw_t[ti][0:rows, sl], upd_ps[key][0:rows, :],
                                                w_t[ti][0:rows, sl].bitcast(F32), op=Alu.add)
                        del upd_ps[key]
                        nc.scalar.dma_start(
                            agj_in[j][tile_base[ti] : tile_base[ti] + rows, :],
                            nw_t[ti][0:rows, sl])
                        done_t[j] = done_t.get(j, 0) + 1
                        if done_t[j] == nt:
                            nc.gpsimd.collective_compute(
                                "AllGather", Alu.bypass, ins=[agj_in[j].opt()],
                                outs=[agj[j].opt()], replica_groups=rg)

                upd_ps, done_t = {}, {}
                DEPTH = 3
                for i in range(nC + DEPTH):
                    if i < nC:
                        emit_mm1(chains[i], i)
                    if 0 <= i - 1 < nC:
                        emit_mm2(chains[i - 1], i)
                    if 0 <= i - DEPTH < nC:
                        emit_mm3(chains[i - DEPTH])

        # ---- phase 3: big matmul per j-block + per-bt in-place softmax
        with tc.tile_pool(name="wnp", bufs=1) as wnp, \
             tc.tile_pool(name="lgp", bufs=1) as lgp, \
             tc.tile_pool(name="smp", bufs=2) as smp, \
             tc.tile_pool(name="p3ps", bufs=1, space="PSUM") as p3ps:
            lg_sb = {}
            mxp = {}
            for jb in range(jt):
                wn_h, wn8_h = [], []
                for hh in range(n_wnh):
                    k0 = hh * KH
                    kn = min(KH, kt_all - k0)
                    wn = wnp.tile([128, kn * 512], F32R, name=f"wn_{jb}_{hh}",
                                  tag="wn", bufs=6)
                    src = agj[jb][k0 * 128 : (k0 + kn) * 128, :].rearrange(
                        "(kt p) m -> p kt m", p=128).bitcast(F32R)
                    nc.sync.dma_start(wn[:].rearrange("p (kt m) -> p kt m", m=512), src)
                    wn_h.append(wn)
                    if xsplit:
                        wn8 = wnp.tile([128, kn * 512], FP8, name=f"wn8_{jb}_{hh}",
                                       tag="wn8", bufs=6)
                        nc.scalar.copy(wn8[:], wn[:].bitcast(F32))
                        wn8_h.append(wn8[:].rearrange("p (t o m) -> p t o m", o=2, m=512))
                for b0 in range(0, bt_all, 2):
                    pair = [b0, b0 + 1] if b0 + 1 < bt_all else [b0]
                    pss = {}
                    for bt in pair:
                        if jb == 0:
                            lg_sb[bt] = lgp.tile([128, M], F32, name=f"lg{jb}_{bt}",
                                                 tag=f"lg{bt}", bufs=1)
                        pss[bt] = p3ps.tile([128, 512], F32, name=f"ps_{jb}_{bt}",
                                            tag="lgps", bufs=6)
                    # interleave the pair so consecutive matmuls alternate PSUM banks
                    for kt in range(kt_all):
                        rhs = wn_h[kt // KH][:, (kt % KH) * 512 : (kt % KH + 1) * 512]
                        for bt in pair:
                            nc.tensor.matmul(pss[bt][:],
                                             xtb_t[bt][:, kt * 128 : (kt + 1) * 128], rhs,
                                             start=(kt == 0),
                                             stop=(kt == kt_all - 1 and not xsplit))
                    if xsplit:
                        kp = kt_all // 2
                        kpH = KH // 2
                        for t in range(kp):
                            rhs8 = wn8_h[t // kpH][:, t % kpH]
                            for bt in pair:
                                xl3d = xtl_t[bt][:].rearrange("p (t o b) -> p t o b",
                                                              o=2, b=128)
                                nc.tensor.matmul(pss[bt][:], xl3d[:, t], rhs8,
                                                 start=False, stop=(t == kp - 1),
                                                 perf_mode=DR)
                    # close the pair: copy to SBUF logits, partial max, softmax
                    for bt in pair:
                        lg = lg_sb[bt]
                        ps = pss[bt]
                        if bt % 2 == 0:
                            nc.scalar.copy(lg[:, jb * 512 : (jb + 1) * 512], ps[:])
                        else:
                            nc.vector.tensor_copy(lg[:, jb * 512 : (jb + 1) * 512], ps[:])
                        if bt not in mxp:
                            mxp[bt] = lgp.tile([128, jt], F32, name=f"mxp{bt}",
                                               tag=f"mxp{bt}")
                        nc.vector.reduce_max(mxp[bt][:, jb : jb + 1], ps[:],
                                             axis=mybir.AxisListType.X)
                        if jb != jt - 1:
                            continue
                        # softmax(bt) in place, overlapping later bts' matmuls
                        mx = smp.tile([128, 1], F32, name=f"mx{bt}", tag="mx")
                        nc.vector.reduce_max(mx[:], mxp[bt][:], axis=mybir.AxisListType.X)
                        nmx = smp.tile([128, 1], F32, name=f"nmx{bt}", tag="nmx")
                        nc.vector.tensor_scalar_mul(nmx[:], mx[:], -1.0)
                        sume = smp.tile([128, 1], F32, name=f"sume{bt}", tag="sume")
                        nc.scalar.activation(lg[:], lg[:], AF.Exp, bias=nmx[:],
                                             accum_out=sume[:])
                        rec = smp.tile([128, 1], F32, name=f"rec{bt}", tag="rec")
                        nc.vector.reciprocal(rec[:], sume[:])
                        nc.vector.tensor_scalar_mul(lg[:], lg[:], rec[:])
                        nc.gpsimd.dma_start(out_d[bt * 128 : (bt + 1) * 128, :], lg[:])
        xp_ctx.__exit__(None, None, None)

    nc.compile()
    meta = dict(B=B, N=N, M=M, n_cores=n_cores, n_shard=n_shard, b_shard=b_shard,
                plan=plan)
    return nc, meta


_CACHE = {}


def _get_program(B, N, M, n_cores, xsplit=True):
    key = (B, N, M, n_cores, xsplit)
    if key not in _CACHE:
        _CACHE[key] = build_program(B, N, M, n_cores, xsplit)
    return _CACHE[key]


def _round_fp32r(x):
    xi = x.view(np.uint32).astype(np.uint64)
    xi = (xi + (1 << 11)) & np.uint64(0xFFFFF000)
    return xi.astype(np.uint32).view(np.float32)


def make_in_maps(meta, consts, X, weight, xsplit=True):
    n_cores, n_shard, b_shard = meta["n_cores"], meta["n_shard"], meta["b_shard"]
    plan = meta["plan"]
    nt = len(plan)
    M = meta["M"]
    N = n_shard * n_cores
    XTp = np.ascontiguousarray(X.T)
    if xsplit:
        XTh = _round_fp32r(XTp)
        XTl8 = (XTp - XTh).astype(ml_dtypes.float8_e5m2)
        XTp = XTh
    base = dict(consts)
    in_maps = []
    for c in range(n_cores):
        m = dict(base)
        rot = np.concatenate([weight[c * n_shard :], weight[: c * n_shard]], axis=0)
        m["wrest16"] = np.ascontiguousarray(
            rot[n_shard:].astype(ml_dtypes.bfloat16).reshape(
                N // 128 - n_shard // 128, 128, M))
        wt = np.zeros((nt, 128, M), dtype=np.float32)
        acc = 0
        for ti, rows in enumerate(plan):
            wt[ti, 0:rows, :] = weight[c * n_shard + acc : c * n_shard + acc + rows, :]
            wt[ti, 127, :] = 1.0
            acc += rows
        m["wsh"] = wt
        m["xtc"] = np.ascontiguousarray(XTp[:, c * b_shard : (c + 1) * b_shard])
        if xsplit:
            m["xtl8"] = np.ascontiguousarray(XTl8[:, c * b_shard : (c + 1) * b_shard])
        in_maps.append(m)
    return in_maps


def run(X, weight, W1, b1, W2, b2, W3, b3, n_cores=8, trace=False, xsplit=True,
        **hw_kwargs):
    X = np.asarray(X, dtype=np.float32)
    weight = np.asarray(weight, dtype=np.float32)
    B, N = X.shape
    M = weight.shape[1]
    nc, meta = _get_program(B, N, M, n_cores, xsplit)
    consts = build_consts(np.asarray(W1, np.float32), np.asarray(b1, np.float32),
                          np.asarray(W2, np.float32), np.asarray(b2, np.float32),
                          np.asarray(W3, np.float32), N, M, meta["n_shard"])
    in_maps = make_in_maps(meta, consts, X, weight, xsplit=xsplit)
    res = bass_utils.run_bass_kernel_spmd(nc, in_maps, core_ids=list(range(n_cores)),
                                          trace=trace, **hw_kwargs)
    out = np.concatenate([res.results[c]["out"] for c in range(n_cores)], axis=0)
    return out, res


def kernel(X, weight, W1, b1, W2, b2, W3, b3):
    xsplit = os.environ.get("BASSNCA_XSPLIT", "1") != "0"
    out, _ = run(X, weight, W1, b1, W2, b2, W3, b3, xsplit=xsplit)
    return out


# revision 26
# speedup vs baseline: 1.1251x; 1.0158x over previous
"""MetaNCA Trainium2 kernel: out = softmax(X @ (W + MLP_percell(W))).

Strategy (8 NeuronCores, SPMD):
  - W row-sharded (256 rows/core) as 126-row tiles; partitions 126/127 hold
    the colsum row and a ones row, so MLP layer 1 is ONE K=128 matmul per
    12-row sub-chunk: pre1 = alpha_c*w + beta_c*colsum_j + (gamma_c*rowsum_i
    + b1_c). Colsum = fp32r masked ones-matmul over the own shard + bf16
    ones-matmuls over the other cores' rows streamed as bf16 (7 MiB) — no
    collective in the critical path (the first-collective fabric wall is
    ~80us here, far more than the bf16 stream).
  - Hidden layers: block-diagonal W2; layer 3 scatters updates tile-wide in
    PSUM; one DVE add produces newW. +b3 dropped (softmax-shift-invariant).
  - MLP chains ordered j-OUTER (columns): each 512-col j-slice of newW
    completes across all tiles, is staged to DRAM and AllGathered immediately
    (jt column-split AGs pipelined under the rest of the MLP + phase 3).
  - Phase 3 single pass per j-block: one [128,512] PSUM per batch-tile
    accumulates fp32r high-X matmuls over 16 k-tiles PLUS the low-X
    correction as 8 fp8e5m2 DoubleRow matmuls (newW cast to fp8 on ACT),
    then lands in SBUF logits; softmax runs in-place per batch-tile right
    after its last j-block, overlapping remaining matmuls. X is
    batch-sharded, host-split into fp32r high + fp8 low parts.
"""

import os
import sys

import numpy as np

for _p in ("/opt/trn_rl_repo", "/root/.axon_site/_ro/trn_rl_repo"):
    if os.path.isdir(_p) and _p not in sys.path:
        sys.path.insert(0, _p)

import ml_dtypes  # noqa: E402

import concourse.bass as bass  # noqa: E402
import concourse.tile as tile  # noqa: E402
from concourse import bacc, bass_utils, mybir  # noqa: E402

F32 = mybir.dt.float32
F32R = mybir.dt.float32r
FP8 = mybir.dt.float8e5
BF16 = mybir.dt.bfloat16
AF = mybir.ActivationFunctionType
Alu = mybir.AluOpType
DR = mybir.MatmulPerfMode.DoubleRow
H = 10
RW = 126  # real W rows per tile (126/127 = colsum/ones)


def _tile_plan(n_shard):
    plan = []
    r = 0
    while r + RW <= n_shard:
        plan.append(RW)
        r += RW
    if r < n_shard:
        plan.append(n_shard - r)
    return plan


def _subchunks(rows):
    subs = []
    r = 0
    while r < rows:
        g = min(12, rows - r)
        subs.append((r, g))
        r += g
    return subs


def build_consts(W1, b1, W2, b2, W3, n, m, n_shard):
    alpha = (W1[0] - W1[1] / np.float32(n - 1) - W1[2] / np.float32(m - 1)).astype(np.float32)
    beta = (W1[1] / np.float32(n - 1)).astype(np.float32)
    gamma = (W1[2] / np.float32(m - 1)).astype(np.float32)
    plan = _tile_plan(n_shard)

    def selb(rows):
        t = np.zeros((128, rows * H), dtype=np.float32)
        for r in range(rows):
            t[r, r * H : (r + 1) * H] = alpha
        t[126, :] = np.tile(beta, rows)
        return t

    def w3sc(rows):
        subs = _subchunks(rows)
        t = np.zeros((120, len(subs) * 128), dtype=np.float32)
        for s, (r0, g_) in enumerate(subs):
            for g in range(g_):
                t[g * H : (g + 1) * H, s * 128 + r0 + g] = W3[:, 0]
        return t

    def blkdiag(mat, g_):
        out = np.zeros((g_ * mat.shape[0], g_ * mat.shape[1]), dtype=np.float32)
        for g in range(g_):
            out[g * mat.shape[0] : (g + 1) * mat.shape[0],
                g * mat.shape[1] : (g + 1) * mat.shape[1]] = mat
        return out

    gset = sorted({g for rows in set(plan) for _, g in _subchunks(rows)})
    c = {
        "ident": np.eye(128, dtype=np.float32),
        "csmaskW": np.concatenate([np.ones(126, np.float32), np.zeros(2, np.float32)])[:, None],
        "ones16": np.ones((128, 1), dtype=ml_dtypes.bfloat16),
    }
    for rows in sorted(set(plan)):
        c[f"selb{rows}"] = selb(rows)
        c[f"w3sc{rows}"] = w3sc(rows)
        c[f"gamT{rows}"] = np.tile(gamma, rows)[None, :]
        c[f"b1T{rows}"] = np.tile(b1, rows)[None, :]
    for g_ in gset:
        c[f"w2b{g_}"] = blkdiag(W2, g_)
        c[f"b2t{g_}"] = np.tile(b2, g_)[:, None].astype(np.float32)
    return c


def build_program(B, N, M, n_cores, xsplit=True):
    n_shard = N // n_cores
    b_shard = B // n_cores
    plan = _tile_plan(n_shard)
    nt = len(plan)
    kt_all = N // 128
    bt_all = b_shard // 128
    jt = M // 512
    kt_rest = kt_all - n_shard // 128
    KH = max(2, kt_all // 4)          # k-tiles per wn sub-tile
    n_wnh = (kt_all + KH - 1) // KH   # wn sub-tiles per j-block

    nc = bacc.Bacc("TRN2", target_bir_lowering=False, debug=False, num_devices=n_cores)

    d = {}
    def din(name, shape, dt):
        d[name] = nc.dram_tensor(name, list(shape), dt, kind="ExternalInput").ap()
    din("wsh", (nt, 128, M), F32R)       # own W shard tiles; row126=0, row127=1
    din("wrest16", (kt_rest, 128, M), BF16)  # other cores' W rows (colsum only)
    din("csmaskW", (128, 1), F32R)
    din("ones16", (128, 1), BF16)
    din("xtc", (N, b_shard), F32R)       # X^T slab (fp32r high part)
    if xsplit:
        din("xtl8", (N, b_shard), FP8)   # fp8 low part of X^T (X - fp32r(X))
    din("ident", (128, 128), F32)
    for rows in sorted(set(plan)):
        din(f"selb{rows}", (128, rows * H), F32R)
        din(f"w3sc{rows}", (120, len(_subchunks(rows)) * 128), F32R)
        din(f"gamT{rows}", (1, rows * H), F32)
        din(f"b1T{rows}", (1, rows * H), F32)
    gset = sorted({g for rows in set(plan) for _, g in _subchunks(rows)})
    for g_ in gset:
        din(f"w2b{g_}", (g_ * H, g_ * H), F32R)
        din(f"b2t{g_}", (g_ * H, 1), F32)
    out_d = nc.dram_tensor("out", [b_shard, M], F32, kind="ExternalOutput").ap()

    rg = [list(range(n_cores))]
    tile_base = []
    acc = 0
    for rows in plan:
        tile_base.append(acc)
        acc += rows

    with tile.TileContext(nc) as tc:
      with tc.tile_pool(name="dram", bufs=1, space="DRAM") as dram:
        agj_in = [dram.tile([n_shard, 512], F32, name=f"agj_in{j}") for j in range(jt)]
        agj = [dram.tile([N, 512], F32, name=f"agj{j}") for j in range(jt)]
        warm_in = dram.tile([1, 16], F32, name="warm_in")
        warm_out = dram.tile([n_cores, 16], F32, name="warm_out")
        with tc.tile_pool(name="wz", bufs=1) as wz:
            wzt = wz.tile([1, 16], F32, name="wzt")
            nc.vector.memset(wzt[:], 0.0)
            nc.sync.dma_start(warm_in[:], wzt[:])
        nc.gpsimd.collective_compute(
            "AllGather", Alu.bypass, ins=[warm_in.opt()], outs=[warm_out.opt()],
            replica_groups=rg)
        xp_ctx = tc.tile_pool(name="xp", bufs=1)
        xp = xp_ctx.__enter__()
        with tc.tile_pool(name="cp", bufs=1) as cp, \
             tc.tile_pool(name="wp", bufs=1) as wp:
            def load(pool, name, dram_ap, shape, dt, eng=None):
                t = pool.tile(shape, dt, name=name)
                (eng or nc.sync).dma_start(t[:], dram_ap[:])
                return t
            # ---- phase 1a: W tiles + full colsum (own fp32r + bf16 stream)
            w_t = []
            for ti, rows in enumerate(plan):
                t = wp.tile([128, M], F32R, name=f"w_t{ti}")
                nc.sync.dma_start(t[:], d["wsh"][ti])
                w_t.append(t)
            csmw_t = load(cp, "csmw_t", d["csmaskW"], [128, 1], F32R, eng=nc.sync)
            ones16_t = load(cp, "ones16_t", d["ones16"], [128, 1], BF16, eng=nc.sync)
            ident_t = load(cp, "ident_t", d["ident"], [128, 128], F32)
            w3_t, w2b_t, b2t_t, gam_t, b1t_t, selbw = {}, {}, {}, {}, {}, []
            for rows in sorted(set(plan)):
                w3_t[rows] = load(cp, f"w3_t{rows}", d[f"w3sc{rows}"],
                                  [120, len(_subchunks(rows)) * 128], F32R)
                gam_t[rows] = load(cp, f"gam_t{rows}", d[f"gamT{rows}"],
                                   [1, rows * H], F32, eng=nc.sync)
                b1t_t[rows] = load(cp, f"b1t_t{rows}", d[f"b1T{rows}"],
                                   [1, rows * H], F32, eng=nc.sync)
            for g_ in gset:
                w2b_t[g_] = load(cp, f"w2b_t{g_}", d[f"w2b{g_}"],
                                 [g_ * H, g_ * H], F32R)
                b2t_t[g_] = load(cp, f"b2t_t{g_}", d[f"b2t{g_}"], [g_ * H, 1], F32,
                                 eng=nc.sync)
            for ti, rows in enumerate(plan):
                st = wp.tile([128, rows * H], F32R, name=f"selbw{ti}", tag=f"selbw{ti}")
                nc.sync.dma_start(st[:], d[f"selb{rows}"][:])
                selbw.append(st)
            with tc.tile_pool(name="p1ps", bufs=1, space="PSUM") as p1ps, \
                 tc.tile_pool(name="p1", bufs=1) as p1, \
                 tc.tile_pool(name="wfp", bufs=1) as wfp:
                def emit_rowsum():
                    rsT_sb = p1.tile([1, nt * 128], F32, name="rsT_sb")
                    for ti in range(nt):
                        rowsum_t = p1.tile([126, 1], F32, name=f"rowsum_{ti}",
                                           tag="rowsum", bufs=2)
                        nc.vector.reduce_sum(rowsum_t[:], w_t[ti][0:126, :].bitcast(F32),
                                             axis=mybir.AxisListType.X)
                        rsT_ps = p1ps.tile([1, 128], F32, name=f"rsT_ps{ti}",
                                           tag="rsT", bufs=2)
                        nc.tensor.transpose(rsT_ps[0:1, 0:126], rowsum_t[:],
                                            ident_t[0:126, 0:126])
                        nc.scalar.copy(rsT_sb[0:1, ti * 128 : ti * 128 + 126],
                                       rsT_ps[0:1, 0:126])
                    for ti, rows in enumerate(plan):
                        r13 = p1.tile([1, rows * H], F32, name=f"r13_{ti}",
                                      tag="r13", bufs=2)
                        rs_b = rsT_sb[0:1, ti * 128 : ti * 128 + rows].unsqueeze(-1) \
                            .broadcast_to([1, rows, H])
                        r3 = r13[:].rearrange("p (n r) -> p n r", r=H)
                        nc.vector.tensor_tensor(
                            r3, rs_b, gam_t[rows][:].rearrange("p (n r) -> p n r", r=H),
                            op=Alu.mult)
                        nc.vector.tensor_tensor(
                            r3, r3, b1t_t[rows][:].rearrange("p (n r) -> p n r", r=H),
                            op=Alu.add)
                        nc.scalar.dma_start(selbw[ti][127:128, :], r13[:].bitcast(F32R))
                CH = 2
                n_ch = (kt_rest + CH - 1) // CH
                wf_t = []
                for c_ in range(n_ch):
                    k0 = c_ * CH
                    kn = min(CH, kt_rest - k0)
                    wf = wfp.tile([128, kn * M], BF16, name=f"wf{c_}", tag="wf", bufs=4)
                    eng = nc.scalar if c_ < max(1, n_ch - 2) else nc.sync
                    src_ap = d["wrest16"].rearrange("t p m -> p t m")[:, k0 : k0 + kn, :]
                    eng.dma_start(wf[:].rearrange("p (t m) -> p t m", m=M), src_ap)
                    wf_t.append((wf, kn))
                colsum_ps = p1ps.tile([1, M], F32, name="colsum_ps")
                for ti in range(nt):
                    for j in range(jt):
                        sl = slice(j * 512, (j + 1) * 512)
                        nc.tensor.matmul(colsum_ps[:, sl], csmw_t[:], w_t[ti][:, sl],
                                         start=(ti == 0), stop=False)
                emit_rowsum()
                for c_, (wf, kn) in enumerate(wf_t):
                    for g in range(kn):
                        for j in range(jt):
                            sl = slice(g * M + j * 512, g * M + (j + 1) * 512)
                            last = (c_ == n_ch - 1 and g == kn - 1 and j == jt - 1)
                            nc.tensor.matmul(colsum_ps[:, j * 512 : (j + 1) * 512],
                                             ones16_t[:], wf[:, sl],
                                             start=False, stop=last)
                # colsum out of PSUM -> stats row of every W tile
                colsum_sb = p1.tile([1, M], F32, name="colsum_sb")
                nc.scalar.copy(colsum_sb[:], colsum_ps[:])
                for ti in range(nt):
                    nc.scalar.dma_start(w_t[ti][126:127, :], colsum_sb[:].bitcast(F32R))
                # warm exp table
                wdum = p1.tile([1, 8], F32, name="wdum")
                nc.vector.memset(wdum[:], 0.0)
                nc.scalar.activation(wdum[:], wdum[:], AF.Exp)

            # X^T prefetch (DMA is otherwise idle during the MLP)
            xtb_t, xtl_t = {}, {}
            def load_x(bt):
                t = xp.tile([128, kt_all * 128], F32R, name=f"xtb{bt}", tag="xtb",
                            bufs=bt_all)
                src = d["xtc"][:, bt * 128 : (bt + 1) * 128].rearrange(
                    "(kt p) b -> p kt b", p=128)
                nc.sync.dma_start(t[:].rearrange("p (kt b) -> p kt b", b=128), src)
                xtb_t[bt] = t
                if xsplit:
                    tl = xp.tile([128, kt_all * 128], FP8, name=f"xtl{bt}", tag="xtl",
                                 bufs=bt_all)
                    srcl = d["xtl8"][:, bt * 128 : (bt + 1) * 128].rearrange(
                        "(kt p) b -> p kt b", p=128)
                    nc.sync.dma_start(tl[:].rearrange("p (kt b) -> p kt b", b=128), srcl)
                    xtl_t[bt] = tl
            for bt in range(bt_all):
                load_x(bt)

            # ---- phase 2: MLP, j-OUTER; AG per completed j-slice
            with tc.tile_pool(name="nwp", bufs=1) as nwp, \
                 tc.tile_pool(name="hp", bufs=6) as hp, \
                 tc.tile_pool(name="p2ps", bufs=1, space="PSUM") as p2ps:
                chains = []
                for j in range(jt):
                    for ti, rows in enumerate(plan):
                        for s, (r0, g_) in enumerate(_subchunks(rows)):
                            chains.append((ti, j, s, r0, g_, rows))
                nC = len(chains)
                state = {}
                nw_t = {}

                def emit_mm1(c, idx):
                    ti, j, s, r0, g_, rows = c
                    sl = slice(j * 512, (j + 1) * 512)
                    Mh = g_ * H
                    ps1 = p2ps.tile([120, 512], F32, name=f"ps1_{ti}_{j}_{s}", tag="ps1", bufs=3)
                    nc.tensor.matmul(ps1[0:Mh, :], selbw[ti][:, r0 * H : r0 * H + Mh],
                                     w_t[ti][:, sl], start=True, stop=True)
                    h1 = hp.tile([120, 512], F32R, name=f"h1_{ti}_{j}_{s}", tag="h1")
                    if idx % 2 == 0:
                        nc.scalar.activation(h1[0:Mh, :], ps1[0:Mh, :], AF.Relu)
                    else:
                        nc.vector.tensor_scalar(h1[0:Mh, :], ps1[0:Mh, :], 0.0, 0.0,
                                                op0=Alu.add, op1=Alu.max)
                    state[c] = (ps1, h1)

                def emit_mm2(c, idx):
                    ti, j, s, r0, g_, rows = c
                    Mh = g_ * H
                    _, h1 = state[c]
                    ps2 = p2ps.tile([120, 512], F32, name=f"ps2_{ti}_{j}_{s}", tag="ps2", bufs=3)
                    nc.tensor.matmul(ps2[0:Mh, :], w2b_t[g_][:], h1[0:Mh, :], start=True, stop=True)
                    h2 = hp.tile([120, 512], F32R, name=f"h2_{ti}_{j}_{s}", tag="h2")
                    if idx % 2 == 0:
                        nc.vector.tensor_scalar(h2[0:Mh, :], ps2[0:Mh, :], b2t_t[g_][0:Mh, :],
                                                0.0, op0=Alu.add, op1=Alu.max)
                    else:
                        nc.scalar.activation(h2[0:Mh, :], ps2[0:Mh, :], AF.Relu,
                                             bias=b2t_t[g_][0:Mh, :])
                    state[c] = (state[c][0], state[c][1], ps2, h2)

                def emit_mm3(c):
                    ti, j, s, r0, g_, rows = c
                    sl = slice(j * 512, (j + 1) * 512)
                    Mh = g_ * H
                    h2 = state.pop(c)[3]
                    subs = _subchunks(rows)
                    key = (ti, j)
                    if key not in upd_ps:
                        upd_ps[key] = p2ps.tile([128, 512], F32, name=f"upd_{ti}_{j}",
                                                tag="upd", bufs=2)
                    nc.tensor.matmul(upd_ps[key][:], w3_t[rows][0:Mh, s * 128 : (s + 1) * 128],
                                     h2[0:Mh, :], start=(s == 0), stop=(s == len(subs) - 1))
                    if s == len(subs) - 1:
                        if ti not in nw_t:
                            nw_t[ti] = nwp.tile([128, M], F32, name=f"nw_t{ti}", tag=f"nw{ti}")
                        nc.vector.tensor_tensor(nw_t[ti][0:rows, sl], upd_ps[key][0:rows, :],
                                                w_t[ti][0:rows, sl].bitcast(F32), op=Alu.add)
                        del upd_ps[key]
                        nc.scalar.dma_start(
                            agj_in[j][tile_base[ti] : tile_base[ti] + rows, :],
                            nw_t[ti][0:rows, sl])
                        done_t[j] = done_t.get(j, 0) + 1
                        if done_t[j] == nt:
                            nc.gpsimd.collective_compute(
                                "AllGather", Alu.bypass, ins=[agj_in[j].opt()],
                                outs=[agj[j].opt()], replica_groups=rg)

                upd_ps, done_t = {}, {}
                DEPTH = 3
                for i in range(nC + DEPTH):
                    if i < nC:
                        emit_mm1(chains[i], i)
                    if 0 <= i - 1 < nC:
                        emit_mm2(chains[i - 1], i)
                    if 0 <= i - DEPTH < nC:
                        emit_mm3(chains[i - DEPTH])

        # ---- phase 3: big matmul per j-block + per-bt in-place softmax
        with tc.tile_pool(name="wnp", bufs=1) as wnp, \
             tc.tile_pool(name="lgp", bufs=1) as lgp, \
             tc.tile_pool(name="smp", bufs=2) as smp, \
             tc.tile_pool(name="p3ps", bufs=1, space="PSUM") as p3ps:
            lg_sb = {}
            mxp = {}
            for jb in range(jt):
                wn_h, wn8_h = [], []
                for hh in range(n_wnh):
                    k0 = hh * KH
                    kn = min(KH, kt_all - k0)
                    wn = wnp.tile([128, kn * 512], F32R, name=f"wn_{jb}_{hh}",
                                  tag="wn", bufs=6)
                    src = agj[jb][k0 * 128 : (k0 + kn) * 128, :].rearrange(
                        "(kt p) m -> p kt m", p=128).bitcast(F32R)
                    nc.sync.dma_start(wn[:].rearrange("p (kt m) -> p kt m", m=512), src)
                    wn_h.append(wn)
                    if xsplit:
                        wn8 = wnp.tile([128, kn * 512], FP8, name=f"wn8_{jb}_{hh}",
                                       tag="wn8", bufs=6)
                        nc.scalar.copy(wn8[:], wn[:].bitcast(F32))
                        wn8_h.append(wn8[:].rearrange("p (t o m) -> p t o m", o=2, m=512))
                for b0 in range(0, bt_all, 2):
                    pair = [b0, b0 + 1] if b0 + 1 < bt_all else [b0]
                    pss = {}
                    for bt in pair:
                        if jb == 0:
                            lg_sb[bt] = lgp.tile([128, M], F32, name=f"lg{jb}_{bt}",
                                                 tag=f"lg{bt}", bufs=1)
                        pss[bt] = p3ps.tile([128, 512], F32, name=f"ps_{jb}_{bt}",
                                            tag="lgps", bufs=6)
                    # interleave the pair so consecutive matmuls alternate PSUM banks
                    for kt in range(kt_all):
                        rhs = wn_h[kt // KH][:, (kt % KH) * 512 : (kt % KH + 1) * 512]
                        for bt in pair:
                            nc.tensor.matmul(pss[bt][:],
                                             xtb_t[bt][:, kt * 128 : (kt + 1) * 128], rhs,
                                             start=(kt == 0),
                                             stop=(kt == kt_all - 1 and not xsplit))
                    if xsplit:
                        kp = kt_all // 2
                        kpH = KH // 2
                        for t in range(kp):
                            rhs8 = wn8_h[t // kpH][:, t % kpH]
                            for bt in pair:
                                xl3d = xtl_t[bt][:].rearrange("p (t o b) -> p t o b",
                                                              o=2, b=128)
                                nc.tensor.matmul(pss[bt][:], xl3d[:, t], rhs8,
                                                 start=False, stop=(t == kp - 1),
                                                 perf_mode=DR)
                    # close the pair: copy to SBUF logits, partial max, softmax
                    for bt in pair:
                        lg = lg_sb[bt]
                        ps = pss[bt]
                        if bt % 2 == 0 and jb != jt - 1:
                            nc.scalar.copy(lg[:, jb * 512 : (jb + 1) * 512], ps[:])
                        else:
                            nc.vector.tensor_copy(lg[:, jb * 512 : (jb + 1) * 512], ps[:])
                        if bt not in mxp:
                            mxp[bt] = lgp.tile([128, jt], F32, name=f"mxp{bt}",
                                               tag=f"mxp{bt}")
                        nc.vector.reduce_max(mxp[bt][:, jb : jb + 1], ps[:],
                                             axis=mybir.AxisListType.X)
                        if jb != jt - 1:
                            continue
                        # softmax(bt) in place, overlapping later bts' matmuls
                        mx = smp.tile([128, 1], F32, name=f"mx{bt}", tag="mx")
                        nc.vector.reduce_max(mx[:], mxp[bt][:], axis=mybir.AxisListType.X)
                        nmx = smp.tile([128, 1], F32, name=f"nmx{bt}", tag="nmx")
                        nc.vector.tensor_scalar_mul(nmx[:], mx[:], -1.0)
                        sume = smp.tile([128, 1], F32, name=f"sume{bt}", tag="sume")
                        nc.scalar.activation(lg[:], lg[:], AF.Exp, bias=nmx[:],
                                             accum_out=sume[:])
                        rec = smp.tile([128, 1], F32, name=f"rec{bt}", tag="rec")
                        nc.vector.reciprocal(rec[:], sume[:])
                        nc.vector.tensor_scalar_mul(lg[:], lg[:], rec[:])
                        nc.gpsimd.dma_start(out_d[bt * 128 : (bt + 1) * 128, :], lg[:])
        xp_ctx.__exit__(None, None, None)

    nc.compile()
    meta = dict(B=B, N=N, M=M, n_cores=n_cores, n_shard=n_shard, b_shard=b_shard,
                plan=plan)
    return nc, meta


_CACHE = {}


def _get_program(B, N, M, n_cores, xsplit=True):
    key = (B, N, M, n_cores, xsplit)
    if key not in _CACHE:
        _CACHE[key] = build_program(B, N, M, n_cores, xsplit)
    return _CACHE[key]


def _round_fp32r(x):
    xi = x.view(np.uint32).astype(np.uint64)
    xi = (xi + (1 << 11)) & np.uint64(0xFFFFF000)
    return xi.astype(np.uint32).view(np.float32)


def make_in_maps(meta, consts, X, weight, xsplit=True):
    n_cores, n_shard, b_shard = meta["n_cores"], meta["n_shard"], meta["b_shard"]
    plan = meta["plan"]
    nt = len(plan)
    M = meta["M"]
    N = n_shard * n_cores
    XTp = np.ascontiguousarray(X.T)
    if xsplit:
        XTh = _round_fp32r(XTp)
        XTl8 = (XTp - XTh).astype(ml_dtypes.float8_e5m2)
        XTp = XTh
    base = dict(consts)
    in_maps = []
    for c in range(n_cores):
        m = dict(base)
        rot = np.concatenate([weight[c * n_shard :], weight[: c * n_shard]], axis=0)
        m["wrest16"] = np.ascontiguousarray(
            rot[n_shard:].astype(ml_dtypes.bfloat16).reshape(
                N // 128 - n_shard // 128, 128, M))
        wt = np.zeros((nt, 128, M), dtype=np.float32)
        acc = 0
        for ti, rows in enumerate(plan):
            wt[ti, 0:rows, :] = weight[c * n_shard + acc : c * n_shard + acc + rows, :]
            wt[ti, 127, :] = 1.0
            acc += rows
        m["wsh"] = wt
        m["xtc"] = np.ascontiguousarray(XTp[:, c * b_shard : (c + 1) * b_shard])
        if xsplit:
            m["xtl8"] = np.ascontiguousarray(XTl8[:, c * b_shard : (c + 1) * b_shard])
        in_maps.append(m)
    return in_maps


def run(X, weight, W1, b1, W2, b2, W3, b3, n_cores=8, trace=False, xsplit=True,
        **hw_kwargs):
    X = np.asarray(X, dtype=np.float32)
    weight = np.asarray(weight, dtype=np.float32)
    B, N = X.shape
    M = weight.shape[1]
    nc, meta = _get_program(B, N, M, n_cores, xsplit)
    consts = build_consts(np.asarray(W1, np.float32), np.asarray(b1, np.float32),
                          np.asarray(W2, np.float32), np.asarray(b2, np.float32),
                          np.asarray(W3, np.float32), N, M, meta["n_shard"])
    in_maps = make_in_maps(meta, consts, X, weight, xsplit=xsplit)
    res = bass_utils.run_bass_kernel_spmd(nc, in_maps, core_ids=list(range(n_cores)),
                                          trace=trace, **hw_kwargs)
    out = np.concatenate([res.results[c]["out"] for c in range(n_cores)], axis=0)
    return out, res


def kernel(X, weight, W1, b1, W2, b2, W3, b3):
    xsplit = os.environ.get("BASSNCA_XSPLIT", "1") != "0"
    out, _ = run(X, weight, W1, b1, W2, b2, W3, b3, xsplit=xsplit)
    return out
